# revision 50
# baseline (speedup 1.0000x reference)
"""Trainium2 Bass kernel for nn_Cov_EBFLayer (v3: padded-triangle s^2 features).

Math: out[b,o] = exp(-quad[o,b]),
  quad[o,b] = diff^T P_o diff,  diff = c_o - x_b,  P_o = B_o B_o^T
            = sum_{d<f} P[d,f] (x_d+x_f)^2            <- s^2 features (2016)
            + sum_d (2 P[d,d] - rowsum_d) x_d^2       <- diag features (64)
            - 2 v_o . x + q3_o                        <- linear (64) + exp bias
  The (x_d+x_f)^2 expansion absorbs the symmetric cross terms; its spurious
  x^2 terms fold into the diagonal coefficients (host, ~5M MACs).

Feature slots live in a GROUP-PADDED triangle space so the whole W reshape
is affine: rows d in [8k, 8k+8) are padded to uniform width w_k = 63-8k
(f in [8k+1, 64)); slots where f <= d are junk (indicator column = 0 so
g = 0 there, junk W x 0 = 0). 2240 padded slots -> 18 s-chunks of 128.

Per core (batch-sharded 8 x 1024):
  - Gram first on PE: 128 pair-Grams row-tiled (even pairs K-rows 0:64, odd
    64:128), PSUM->SBUF strided copies (ACT j=0 / DVE j=1) into
    p_sb[f1, (f2, o')], o' = j*128 + r*64 + m (4-contiguous per copy).
  - W path: 8 affine group-writes p_sb->p_dram[2304,256], 4 chunked affine
    reads -> W[slot, (chunk, o')].  Chunk 0 of W comes from host (diag+lin).
  - s-chunks: s = indc-matmul (two-hot, K=64, base-partition alternating),
    g = Square(s): ACT direct (bt0) or DVE copy+mul (bt1).
  - mains: per o-half (serial, 2 PSUM banks): 20 accumulating matmuls per
    b-tile; epilogue EXP(scale=-1, bias=-q3[o']) -> fp16 out.
"""

import sys
from contextlib import ExitStack

import numpy as np

sys.path.insert(0, "/opt/trn_rl_repo")

import concourse.bass as bass  # noqa: E402
import concourse.tile as tile  # noqa: E402
from concourse import bacc, mybir  # noqa: E402
from concourse import bass_utils  # noqa: E402
from concourse._compat import with_exitstack  # noqa: E402

B, D, O, NCORES = 8192, 64, 256, 8
BSH = B // NCORES  # 1024 per-core batch shard
BT = 512
NBT = BSH // BT  # 2
NSC = 18  # s^2 chunks over the 2240-slot padded triangle
NCHUNK = NSC + 1  # + diag/linear chunk 0
NPAD = 2240
F32 = mybir.dt.float32
F16 = mybir.dt.float16

_GBASE = []  # padded-row base per group of 8 d-rows
_GW = []
_b = 0
for _k in range(8):
    _w = 63 - 8 * _k
    _GBASE.append(_b)
    _GW.append(_w)
    _b += 8 * _w


def _slot_df(sl):
    """padded slot -> (d, f) or None if junk."""
    for k in range(8):
        if sl < _GBASE[k] + 8 * _GW[k]:
            off = sl - _GBASE[k]
            d = 8 * k + off // _GW[k]
            f = 8 * k + 1 + off % _GW[k]
            return (d, f) if f > d else None
    return None


@with_exitstack
def _kernel(ctx: ExitStack, tc, outT, xT, betasT, indc, linW, negq3):
    nc = tc.nc

    cpool = ctx.enter_context(tc.tile_pool(name="const", bufs=1))
    dpool = ctx.enter_context(tc.tile_pool(name="dram", bufs=1, space="DRAM"))
    ge_pool = ctx.enter_context(tc.tile_pool(name="psum_ge", bufs=3, space="PSUM"))
    go_pool = ctx.enter_context(tc.tile_pool(name="psum_go", bufs=3, space="PSUM"))
    s_pool = ctx.enter_context(tc.tile_pool(name="psum_s", bufs=2, space="PSUM"))
    opool = ctx.enter_context(tc.tile_pool(name="outs", bufs=4))

    # ---- inputs (spread over sync/scalar/gpsimd rings to unblock PE) ----
    xb = cpool.tile([128, BSH], F16)  # [xT; xT]
    nc.sync.dma_start(xb[0:D, :], xT[:])
    nc.scalar.dma_start(xb[D:128, :], xT[:])
    sb_indc = cpool.tile([128, 9 * 128], F16)
    nc.sync.dma_start(sb_indc[:], indc[:])
    w = cpool.tile([128, NCHUNK * 256], F16)  # [slot, (chunk, o')]
    nc.scalar.dma_start(w[:, 0:256], linW[:])  # chunk 0 from host
    sb_nq3 = cpool.tile([128, 2], F32)
    nc.scalar.dma_start(sb_nq3[:], negq3[:])
    sb_betasT = cpool.tile([128, 64 * 128], F16)  # row r: pairs tt%2==r
    nc.sync.dma_start(sb_betasT[:, 0:2048], betasT[:, 0:2048])
    for k in range(1, 4):
        nc.gpsimd.dma_start(
            sb_betasT[:, k * 2048 : (k + 1) * 2048],
            betasT[:, k * 2048 : (k + 1) * 2048],
        )

    # ---- g chunks ----
    g = [
        cpool.tile([128, BSH], F16, name=f"g{c}", uniquify=False)
        for c in range(NCHUNK)
    ]
    # chunk 0: [x^2 ; x]
    nc.scalar.dma_start(g[0][D:128, :], xT[:])
    nc.vector.tensor_mul(g[0][0:D, :], xb[0:D, :], xb[0:D, :])

    # PE warm-up: junk matmuls gated only on the first xb DMA (results are
    # overwritten by the real Gram accumulations via start=True)
    wu = ge_pool.tile([128, 512], F32, name="wu", tag="pp0")
    for i in range(4):
        nc.tensor.matmul(
            wu[:, (i % 4) * 128 : (i % 4 + 1) * 128],
            xb[0:D, 0:128],
            xb[0:D, 0:128],
            start=True,
            stop=True,
        )

    def emit_s_chunk(c):
        b = (c - 1) // 2
        r0 = 0 if c % 2 == 1 else 64
        for bt in range(NBT):
            ps = s_pool.tile([128, BT], F32, name=f"s_{c}_{bt}", tag="s")
            nc.tensor.matmul(
                ps[:],
                sb_indc[r0 : r0 + 64, b * 128 : (b + 1) * 128],
                xb[r0 : r0 + 64, bt * BT : (bt + 1) * BT],
                start=True,
                stop=True,
            )
            if bt == 0 or c >= 14:
                nc.scalar.activation(
                    g[c][:, bt * BT : (bt + 1) * BT],
                    ps[:],
                    mybir.ActivationFunctionType.Square,
                )
            else:
                tmp = opool.tile([128, BT], F16, name=f"st_{c}", tag="st")
                nc.vector.tensor_copy(tmp[:], ps[:])
                nc.vector.tensor_mul(
                    g[c][:, bt * BT : (bt + 1) * BT], tmp[:], tmp[:]
                )

    p_sb = cpool.tile([D, D * 256], F16)  # [f1, (f2, o')]

    def emit_gram_group(gidx):
        for r in range(2):  # tile-row: even/odd pairs
            pool = ge_pool if r == 0 else go_pool
            pp = pool.tile([128, 512], F32, name=f"pp_{gidx}_{r}", tag=f"pp{r}")
            for q in range(4):
                m = gidx * 4 + q
                blk = sb_betasT[r * 64 : r * 64 + 64, m * 128 : (m + 1) * 128]
                nc.tensor.matmul(
                    pp[:, q * 128 : (q + 1) * 128], blk, blk, start=True, stop=True
                )
            # copies: src partitions (j, f1), free (f2 of j2==j, q);
            # dst p_sb[f1, f2*256 + (j*128 + r*64 + 4*gidx + q)]
            for j in range(2):
                src = pp[j * 64 : (j + 1) * 64, :].rearrange(
                    "p (q j2 f) -> p j2 f q", q=4, j2=2
                )[:, j, :, :]
                dstv = p_sb[:].rearrange(
                    "p (f j r gi q) -> p j r gi f q", j=2, r=2, gi=16, q=4
                )
                dst = dstv[:, j, r, gidx, :, :]
                if j == 0:
                    nc.scalar.activation(
                        dst, src, mybir.ActivationFunctionType.Copy
                    )
                else:
                    nc.vector.tensor_copy(dst, src)

    # ---- emission: 2 s-chunks as cold warm-up, all Gram, W path, rest of s ----
    emit_s_chunk(1)
    emit_s_chunk(2)
    for gidx in range(16):
        emit_gram_group(gidx)

    # W path: 8 affine padded group-writes (sync/gpsimd alternating so two
    # transfers fly at once; gpsimd SWDGE keeps the ACT ring unblocked),
    # reads interleaved on sync as soon as their groups land.
    p_dram = dpool.tile([2304, 256], F16)  # [padded-slot, o']
    p_sb_v = p_sb[:].rearrange("p (f o) -> p f o", o=256)
    wv = w[:, 256:].rearrange("p (c o) -> p c o", o=256)

    def wwrite(k):
        # SWDGE for the wide groups (transfers drain in parallel; each
        # group's 8 partitions map to a distinct SDMA-engine pair); the
        # two narrow tail groups ride the idle sync ring.
        eng = nc.gpsimd if k < 6 else nc.sync
        eng.dma_start(
            p_dram[_GBASE[k] : _GBASE[k] + 8 * _GW[k], :],
            p_sb_v[8 * k : 8 * k + 8, 8 * k + 1 : D, :],
        )

    def wread(i):
        rd_bounds = [0, 2, 6, 12, 18]
        c0, c1 = rd_bounds[i], rd_bounds[i + 1]
        eng = nc.sync if i % 2 == 0 else nc.gpsimd
        eng.dma_start(
            wv[:, c0:c1, :],
            p_dram[c0 * 128 : c1 * 128, :].rearrange("(c p) o -> p c o", p=128),
        )

    # read deps: r0<-g0; r1<-g0,g1; r2<-g1..g4; r3<-g4..g7
    wwrite(0)
    wread(0)
    wwrite(1)
    wread(1)
    wwrite(2)
    wwrite(3)
    wwrite(4)
    wread(2)
    wwrite(5)
    wwrite(6)
    wwrite(7)
    wread(3)

    for c in range(3, NSC + 1):
        emit_s_chunk(c)

    # ---- mains: 4 concurrent accumulation chains (pq banks from the gram
    # pools -- their rotation frees after the last gram copies) ----
    pq = {}
    for oh in range(2):
        for bt in range(NBT):
            pool = ge_pool if bt == 0 else go_pool
            pq[(oh, bt)] = pool.tile(
                [128, BT], F32, name=f"pq_{oh}_{bt}", tag=f"pp{0 if bt == 0 else 1}"
            )
    for c in range(NCHUNK):
        kc = 64 if c == NSC else 128
        for oh in range(2):
            for bt in range(NBT):
                nc.tensor.matmul(
                    pq[(oh, bt)][:],
                    w[0:kc, c * 256 + oh * 128 : c * 256 + oh * 128 + 128],
                    g[c][0:kc, bt * BT : (bt + 1) * BT],
                    start=(c == 0),
                    stop=(c == NCHUNK - 1),
                )
    for oh in range(2):
        for bt in range(NBT):
            ob = opool.tile([128, BT], F16, name=f"ob_{oh}_{bt}", tag="ob")
            nc.scalar.activation(
                ob[:],
                pq[(oh, bt)][:],
                mybir.ActivationFunctionType.Exp,
                scale=-1.0,
                bias=sb_nq3[:, oh : oh + 1],
            )
            oeng = nc.sync if bt == 0 else nc.scalar
            oeng.dma_start(
                outT[oh * 128 : (oh + 1) * 128, bt * BT : (bt + 1) * BT], ob[:]
            )


_CACHE = {}


def _build():
    if "nc" in _CACHE:
        return _CACHE["nc"], _CACHE["aps"]
    nc = bacc.Bacc(
        "TRN2", target_bir_lowering=False, debug=False, num_devices=NCORES
    )
    xT = nc.dram_tensor("xT", [D, BSH], F16, kind="ExternalInput").ap()
    betasT = nc.dram_tensor("betasT", [128, 64 * 128], F16, kind="ExternalInput").ap()
    indc = nc.dram_tensor("indc", [128, 9 * 128], F16, kind="ExternalInput").ap()
    linW = nc.dram_tensor("linW", [128, 256], F16, kind="ExternalInput").ap()
    negq3 = nc.dram_tensor("negq3", [128, 2], F32, kind="ExternalInput").ap()
    outT = nc.dram_tensor("outT", [O, BSH], F16, kind="ExternalOutput").ap()
    with tile.TileContext(nc) as tc:
        _kernel(tc, outT, xT, betasT, indc, linW, negq3)
    nc.compile()
    _CACHE["nc"] = nc
    _CACHE["aps"] = (xT, betasT, indc, linW, negq3, outT)
    return nc, _CACHE["aps"]


def _operm():
    # o' = j*128 + r*64 + m,  j = o%2, tt = o//2, r = tt%2, m = tt//2
    o = np.arange(O)
    tt = o // 2
    return (o % 2) * 128 + (tt % 2) * 64 + tt // 2


def _host_prep(x, centers, betas):
    x = np.asarray(x, np.float32)
    betas = np.asarray(betas, np.float32)
    cen = np.asarray(centers, np.float32).reshape(O, D)
    operm = _operm()

    # betasT stacked for row-tiled Gram: row-block r holds pairs tt%2==r;
    # block m of row r covers pair tt=2m+r -> o = (4m+2r, 4m+2r+1)
    bt_all = betas.transpose(2, 0, 1)  # [e, o, f]
    ev = np.stack([bt_all[:, 4 * m : 4 * m + 2, :] for m in range(64)], axis=1)
    od = np.stack([bt_all[:, 4 * m + 2 : 4 * m + 4, :] for m in range(64)], axis=1)
    betasT = np.concatenate(
        [ev.reshape(D, 64 * 128), od.reshape(D, 64 * 128)], axis=0
    ).astype(np.float16)

    # indc: two-hot columns for real padded slots, zero for junk
    indc = np.zeros((128, 9 * 128), np.float32)
    for cc in range(1, NSC + 1):
        blk = (cc - 1) // 2
        r0 = 0 if cc % 2 == 1 else 64
        for p in range(128):
            sl = (cc - 1) * 128 + p
            if sl >= NPAD:
                break
            df = _slot_df(sl)
            if df is None:
                continue
            d, f = df
            indc[r0 + d, blk * 128 + p] = 1.0
            indc[r0 + f, blk * 128 + p] = 1.0
    indc = indc.astype(np.float16)

    # host linear prep (~5M MACs)
    s1 = betas.sum(axis=1)  # [O, e]
    rowsum = np.einsum("ode,oe->od", betas, s1)
    pdd = (betas ** 2).sum(axis=2)
    diagW = 2.0 * pdd - rowsum
    wvec = np.einsum("ofe,of->oe", betas, cen)
    v = np.einsum("ode,oe->od", betas, wvec)
    q3 = (wvec ** 2).sum(axis=1)

    linW = np.zeros((128, 256), np.float32)
    linW[0:D, operm] = diagW.T
    linW[D:128, operm] = (-2.0 * v).T
    linW = linW.astype(np.float16)

    negq3 = np.zeros((128, 2), np.float32)
    inv = np.empty(O, np.int64)
    inv[operm] = np.arange(O)
    for oh in range(2):
        negq3[:, oh] = -q3[inv[oh * 128 : (oh + 1) * 128]]

    xT_shards = [
        np.ascontiguousarray(x[i * BSH : (i + 1) * BSH].T).astype(np.float16)
        for i in range(NCORES)
    ]
    return xT_shards, betasT, indc, linW, negq3


def _run(x, centers, betas, trace=False):
    nc, (xT, betasT_ap, indc_ap, linW_ap, negq3_ap, outT) = _build()
    xT_shards, betasT, indc, linW, negq3 = _host_prep(x, centers, betas)
    in_maps = [
        {
            xT.name: xT_shards[i],
            betasT_ap.name: betasT,
            indc_ap.name: indc,
            linW_ap.name: linW,
            negq3_ap.name: negq3,
        }
        for i in range(NCORES)
    ]
    res = bass_utils.run_bass_kernel_spmd(
        nc, in_maps, core_ids=list(range(NCORES)), trace=trace
    )
    operm = _operm()
    out = np.concatenate(
        [
            np.asarray(res.results[i][outT.name])[operm, :].T.astype(np.float32)
            for i in range(NCORES)
        ],
        axis=0,
    )
    return out, res


def kernel(x, centers, betas):
    out, _ = _run(x, centers, betas, trace=False)
    return out


# revision 51
# speedup vs baseline: 1.0432x; 1.0432x over previous
"""Trainium2 Bass kernel for nn_Cov_EBFLayer (v3: padded-triangle s^2 features).

Math: out[b,o] = exp(-quad[o,b]),
  quad[o,b] = diff^T P_o diff,  diff = c_o - x_b,  P_o = B_o B_o^T
            = sum_{d<f} P[d,f] (x_d+x_f)^2            <- s^2 features (2016)
            + sum_d (2 P[d,d] - rowsum_d) x_d^2       <- diag features (64)
            - 2 v_o . x + q3_o                        <- linear (64) + exp bias
  The (x_d+x_f)^2 expansion absorbs the symmetric cross terms; its spurious
  x^2 terms fold into the diagonal coefficients (host, ~5M MACs).

Feature slots live in a GROUP-PADDED triangle space so the whole W reshape
is affine: rows d in [8k, 8k+8) are padded to uniform width w_k = 63-8k
(f in [8k+1, 64)); slots where f <= d are junk (indicator column = 0 so
g = 0 there, junk W x 0 = 0). 2240 padded slots -> 18 s-chunks of 128.

Per core (batch-sharded 8 x 1024):
  - Gram first on PE: 128 pair-Grams row-tiled (even pairs K-rows 0:64, odd
    64:128), PSUM->SBUF strided copies (ACT j=0 / DVE j=1) into
    p_sb[f1, (f2, o')], o' = j*128 + r*64 + m (4-contiguous per copy).
  - W path: 8 affine group-writes p_sb->p_dram[2304,256], 4 chunked affine
    reads -> W[slot, (chunk, o')].  Chunk 0 of W comes from host (diag+lin).
  - s-chunks: s = indc-matmul (two-hot, K=64, base-partition alternating),
    g = Square(s): ACT direct (bt0) or DVE copy+mul (bt1).
  - mains: per o-half (serial, 2 PSUM banks): 20 accumulating matmuls per
    b-tile; epilogue EXP(scale=-1, bias=-q3[o']) -> fp16 out.
"""

import sys
from contextlib import ExitStack

import numpy as np

sys.path.insert(0, "/opt/trn_rl_repo")

import concourse.bass as bass  # noqa: E402
import concourse.tile as tile  # noqa: E402
from concourse import bacc, mybir  # noqa: E402
from concourse import bass_utils  # noqa: E402
from concourse._compat import with_exitstack  # noqa: E402

B, D, O, NCORES = 8192, 64, 256, 8
BSH = B // NCORES  # 1024 per-core batch shard
BT = 512
NBT = BSH // BT  # 2
NSC = 18  # s^2 chunks over the 2240-slot padded triangle
NCHUNK = NSC + 1  # + diag/linear chunk 0
NPAD = 2240
F32 = mybir.dt.float32
F16 = mybir.dt.float16

_GBASE = []  # padded-row base per group of 8 d-rows
_GW = []
_b = 0
for _k in range(8):
    _w = 63 - 8 * _k
    _GBASE.append(_b)
    _GW.append(_w)
    _b += 8 * _w


def _slot_df(sl):
    """padded slot -> (d, f) or None if junk."""
    for k in range(8):
        if sl < _GBASE[k] + 8 * _GW[k]:
            off = sl - _GBASE[k]
            d = 8 * k + off // _GW[k]
            f = 8 * k + 1 + off % _GW[k]
            return (d, f) if f > d else None
    return None


@with_exitstack
def _kernel(ctx: ExitStack, tc, outT, xT, betasT, indc, linW, negq3):
    nc = tc.nc

    cpool = ctx.enter_context(tc.tile_pool(name="const", bufs=1))
    dpool = ctx.enter_context(tc.tile_pool(name="dram", bufs=1, space="DRAM"))
    ge_pool = ctx.enter_context(tc.tile_pool(name="psum_ge", bufs=3, space="PSUM"))
    go_pool = ctx.enter_context(tc.tile_pool(name="psum_go", bufs=3, space="PSUM"))
    s_pool = ctx.enter_context(tc.tile_pool(name="psum_s", bufs=2, space="PSUM"))
    opool = ctx.enter_context(tc.tile_pool(name="outs", bufs=4))

    # ---- inputs (spread over sync/scalar/gpsimd rings to unblock PE) ----
    xb = cpool.tile([128, BSH], F16)  # [xT; xT]
    nc.sync.dma_start(xb[0:D, :], xT[:])
    nc.scalar.dma_start(xb[D:128, :], xT[:])
    sb_indc = cpool.tile([128, 9 * 128], F16)
    nc.sync.dma_start(sb_indc[:], indc[:])
    w = cpool.tile([128, NCHUNK * 256], F16)  # [slot, (chunk, o')]
    nc.scalar.dma_start(w[:, 0:256], linW[:])  # chunk 0 from host
    sb_nq3 = cpool.tile([128, 2], F32)
    nc.scalar.dma_start(sb_nq3[:], negq3[:])
    sb_betasT = cpool.tile([128, 64 * 128], F16)  # row r: pairs tt%2==r
    nc.sync.dma_start(sb_betasT[:, 0:2048], betasT[:, 0:2048])
    for k in range(1, 4):
        nc.gpsimd.dma_start(
            sb_betasT[:, k * 2048 : (k + 1) * 2048],
            betasT[:, k * 2048 : (k + 1) * 2048],
        )

    # ---- g chunks ----
    g = [
        cpool.tile([128, BSH], F16, name=f"g{c}", uniquify=False)
        for c in range(NCHUNK)
    ]
    # chunk 0: [x^2 ; x]
    nc.scalar.dma_start(g[0][D:128, :], xT[:])
    nc.vector.tensor_mul(g[0][0:D, :], xb[0:D, :], xb[0:D, :])

    # PE warm-up: junk matmuls gated only on the first xb DMA (results are
    # overwritten by the real Gram accumulations via start=True)
    wu = ge_pool.tile([128, 512], F32, name="wu", tag="pp0")
    for i in range(4):
        nc.tensor.matmul(
            wu[:, (i % 4) * 128 : (i % 4 + 1) * 128],
            xb[0:D, 0:128],
            xb[0:D, 0:128],
            start=True,
            stop=True,
        )

    def emit_s_chunk(c):
        b = (c - 1) // 2
        r0 = 0 if c % 2 == 1 else 64
        for bt in range(NBT):
            ps = s_pool.tile([128, BT], F32, name=f"s_{c}_{bt}", tag="s")
            nc.tensor.matmul(
                ps[:],
                sb_indc[r0 : r0 + 64, b * 128 : (b + 1) * 128],
                xb[r0 : r0 + 64, bt * BT : (bt + 1) * BT],
                start=True,
                stop=True,
            )
            if bt == 0:
                nc.scalar.activation(
                    g[c][:, bt * BT : (bt + 1) * BT],
                    ps[:],
                    mybir.ActivationFunctionType.Square,
                )
            else:
                tmp = opool.tile([128, BT], F16, name=f"st_{c}", tag="st")
                nc.vector.tensor_copy(tmp[:], ps[:])
                nc.vector.tensor_mul(
                    g[c][:, bt * BT : (bt + 1) * BT], tmp[:], tmp[:]
                )

    p_sb = cpool.tile([D, D * 256], F16)  # [f1, (f2, o')]

    def emit_gram_group(gidx):
        for r in range(2):  # tile-row: even/odd pairs
            pool = ge_pool if r == 0 else go_pool
            pp = pool.tile([128, 512], F32, name=f"pp_{gidx}_{r}", tag=f"pp{r}")
            for q in range(4):
                m = gidx * 4 + q
                blk = sb_betasT[r * 64 : r * 64 + 64, m * 128 : (m + 1) * 128]
                nc.tensor.matmul(
                    pp[:, q * 128 : (q + 1) * 128], blk, blk, start=True, stop=True
                )
            # copies: src partitions (j, f1), free (f2 of j2==j, q);
            # dst p_sb[f1, f2*256 + (j*128 + r*64 + 4*gidx + q)]
            for j in range(2):
                src = pp[j * 64 : (j + 1) * 64, :].rearrange(
                    "p (q j2 f) -> p j2 f q", q=4, j2=2
                )[:, j, :, :]
                dstv = p_sb[:].rearrange(
                    "p (f j r gi q) -> p j r gi f q", j=2, r=2, gi=16, q=4
                )
                dst = dstv[:, j, r, gidx, :, :]
                if j == 0:
                    nc.scalar.activation(
                        dst, src, mybir.ActivationFunctionType.Copy
                    )
                else:
                    nc.vector.tensor_copy(dst, src)

    # ---- emission: 2 s-chunks as cold warm-up, all Gram, W path, rest of s ----
    emit_s_chunk(1)
    emit_s_chunk(2)
    for gidx in range(16):
        emit_gram_group(gidx)

    # W path: 8 affine padded group-writes (sync/gpsimd alternating so two
    # transfers fly at once; gpsimd SWDGE keeps the ACT ring unblocked),
    # reads interleaved on sync as soon as their groups land.
    p_dram = dpool.tile([2304, 256], F16)  # [padded-slot, o']
    p_sb_v = p_sb[:].rearrange("p (f o) -> p f o", o=256)
    wv = w[:, 256:].rearrange("p (c o) -> p c o", o=256)

    def wwrite(k):
        # SWDGE for the wide groups (transfers drain in parallel; each
        # group's 8 partitions map to a distinct SDMA-engine pair); the
        # two narrow tail groups ride the idle sync ring.
        eng = nc.gpsimd if k < 6 else nc.sync
        eng.dma_start(
            p_dram[_GBASE[k] : _GBASE[k] + 8 * _GW[k], :],
            p_sb_v[8 * k : 8 * k + 8, 8 * k + 1 : D, :],
        )

    def wread(i):
        rd_bounds = [0, 2, 6, 12, 18]
        c0, c1 = rd_bounds[i], rd_bounds[i + 1]
        eng = nc.sync if i % 2 == 0 else nc.gpsimd
        eng.dma_start(
            wv[:, c0:c1, :],
            p_dram[c0 * 128 : c1 * 128, :].rearrange("(c p) o -> p c o", p=128),
        )

    # read deps: r0<-g0; r1<-g0,g1; r2<-g1..g4; r3<-g4..g7
    wwrite(0)
    wread(0)
    wwrite(1)
    wread(1)
    wwrite(2)
    wwrite(3)
    wwrite(4)
    wread(2)
    wwrite(5)
    wwrite(6)
    wwrite(7)
    wread(3)

    for c in range(3, NSC + 1):
        emit_s_chunk(c)

    # ---- mains: 4 concurrent accumulation chains (pq banks from the gram
    # pools -- their rotation frees after the last gram copies) ----
    pq = {}
    for oh in range(2):
        for bt in range(NBT):
            pool = ge_pool if bt == 0 else go_pool
            pq[(oh, bt)] = pool.tile(
                [128, BT], F32, name=f"pq_{oh}_{bt}", tag=f"pp{0 if bt == 0 else 1}"
            )
    for c in range(NCHUNK):
        kc = 64 if c == NSC else 128
        for oh in range(2):
            for bt in range(NBT):
                nc.tensor.matmul(
                    pq[(oh, bt)][:],
                    w[0:kc, c * 256 + oh * 128 : c * 256 + oh * 128 + 128],
                    g[c][0:kc, bt * BT : (bt + 1) * BT],
                    start=(c == 0),
                    stop=(c == NCHUNK - 1),
                )
    for oh in range(2):
        for bt in range(NBT):
            ob = opool.tile([128, BT], F16, name=f"ob_{oh}_{bt}", tag="ob")
            nc.scalar.activation(
                ob[:],
                pq[(oh, bt)][:],
                mybir.ActivationFunctionType.Exp,
                scale=-1.0,
                bias=sb_nq3[:, oh : oh + 1],
            )
            oeng = nc.sync if bt == 0 else nc.scalar
            oeng.dma_start(
                outT[oh * 128 : (oh + 1) * 128, bt * BT : (bt + 1) * BT], ob[:]
            )


_CACHE = {}


def _build():
    if "nc" in _CACHE:
        return _CACHE["nc"], _CACHE["aps"]
    nc = bacc.Bacc(
        "TRN2", target_bir_lowering=False, debug=False, num_devices=NCORES
    )
    xT = nc.dram_tensor("xT", [D, BSH], F16, kind="ExternalInput").ap()
    betasT = nc.dram_tensor("betasT", [128, 64 * 128], F16, kind="ExternalInput").ap()
    indc = nc.dram_tensor("indc", [128, 9 * 128], F16, kind="ExternalInput").ap()
    linW = nc.dram_tensor("linW", [128, 256], F16, kind="ExternalInput").ap()
    negq3 = nc.dram_tensor("negq3", [128, 2], F32, kind="ExternalInput").ap()
    outT = nc.dram_tensor("outT", [O, BSH], F16, kind="ExternalOutput").ap()
    with tile.TileContext(nc) as tc:
        _kernel(tc, outT, xT, betasT, indc, linW, negq3)
    nc.compile()
    _CACHE["nc"] = nc
    _CACHE["aps"] = (xT, betasT, indc, linW, negq3, outT)
    return nc, _CACHE["aps"]


def _operm():
    # o' = j*128 + r*64 + m,  j = o%2, tt = o//2, r = tt%2, m = tt//2
    o = np.arange(O)
    tt = o // 2
    return (o % 2) * 128 + (tt % 2) * 64 + tt // 2


def _host_prep(x, centers, betas):
    x = np.asarray(x, np.float32)
    betas = np.asarray(betas, np.float32)
    cen = np.asarray(centers, np.float32).reshape(O, D)
    operm = _operm()

    # betasT stacked for row-tiled Gram: row-block r holds pairs tt%2==r;
    # block m of row r covers pair tt=2m+r -> o = (4m+2r, 4m+2r+1)
    bt_all = betas.transpose(2, 0, 1)  # [e, o, f]
    ev = np.stack([bt_all[:, 4 * m : 4 * m + 2, :] for m in range(64)], axis=1)
    od = np.stack([bt_all[:, 4 * m + 2 : 4 * m + 4, :] for m in range(64)], axis=1)
    betasT = np.concatenate(
        [ev.reshape(D, 64 * 128), od.reshape(D, 64 * 128)], axis=0
    ).astype(np.float16)

    # indc: two-hot columns for real padded slots, zero for junk
    indc = np.zeros((128, 9 * 128), np.float32)
    for cc in range(1, NSC + 1):
        blk = (cc - 1) // 2
        r0 = 0 if cc % 2 == 1 else 64
        for p in range(128):
            sl = (cc - 1) * 128 + p
            if sl >= NPAD:
                break
            df = _slot_df(sl)
            if df is None:
                continue
            d, f = df
            indc[r0 + d, blk * 128 + p] = 1.0
            indc[r0 + f, blk * 128 + p] = 1.0
    indc = indc.astype(np.float16)

    # host linear prep (~5M MACs)
    s1 = betas.sum(axis=1)  # [O, e]
    rowsum = np.einsum("ode,oe->od", betas, s1)
    pdd = (betas ** 2).sum(axis=2)
    diagW = 2.0 * pdd - rowsum
    wvec = np.einsum("ofe,of->oe", betas, cen)
    v = np.einsum("ode,oe->od", betas, wvec)
    q3 = (wvec ** 2).sum(axis=1)

    linW = np.zeros((128, 256), np.float32)
    linW[0:D, operm] = diagW.T
    linW[D:128, operm] = (-2.0 * v).T
    linW = linW.astype(np.float16)

    negq3 = np.zeros((128, 2), np.float32)
    inv = np.empty(O, np.int64)
    inv[operm] = np.arange(O)
    for oh in range(2):
        negq3[:, oh] = -q3[inv[oh * 128 : (oh + 1) * 128]]

    xT_shards = [
        np.ascontiguousarray(x[i * BSH : (i + 1) * BSH].T).astype(np.float16)
        for i in range(NCORES)
    ]
    return xT_shards, betasT, indc, linW, negq3


def _run(x, centers, betas, trace=False):
    nc, (xT, betasT_ap, indc_ap, linW_ap, negq3_ap, outT) = _build()
    xT_shards, betasT, indc, linW, negq3 = _host_prep(x, centers, betas)
    in_maps = [
        {
            xT.name: xT_shards[i],
            betasT_ap.name: betasT,
            indc_ap.name: indc,
            linW_ap.name: linW,
            negq3_ap.name: negq3,
        }
        for i in range(NCORES)
    ]
    res = bass_utils.run_bass_kernel_spmd(
        nc, in_maps, core_ids=list(range(NCORES)), trace=trace
    )
    operm = _operm()
    out = np.concatenate(
        [
            np.asarray(res.results[i][outT.name])[operm, :].T.astype(np.float32)
            for i in range(NCORES)
        ],
        axis=0,
    )
    return out, res


def kernel(x, centers, betas):
    out, _ = _run(x, centers, betas, trace=False)
    return out
